# revision 17
# baseline (speedup 1.0000x reference)
"""GQA attention (16 Q heads / 4 KV heads, head_dim 128, RoPE, varlen causal)
on 8 Trainium2 NeuronCores, tensor-parallel over heads.

Per core c: Q heads {2c, 2c+1}, KV head c//2.
Pipeline: QKV projection (bf16 matmul, f32 PSUM) -> RoPE (permutation-matmul
swap + DVE combine) -> block-sparse S^T-layout attention (exp on ScalarE,
softmax denominators via ones-matmul, PV accumulated directly in O^T layout)
-> normalize (fast DVE reciprocal) -> AllToAll (each core receives the full
attention output for its 512-token slice) -> output projection on that slice.

Host-side prep: x shipped pre-transposed; 1/sqrt(HD) folded into wq; wq/wk
columns permuted per head so RoPE's interleaved pairs become [evens | odds]
(dot products are permutation-invariant); varlen-causal mask block structure
computed from seq_ids and baked into the (shared, SPMD) program, with
multiplicative {0,1} masks shipped only for partially-masked blocks.
"""
import os
import sys

for _p in ("/opt/trn_rl_repo",):
    if _p not in sys.path:
        sys.path.insert(0, _p)

import numpy as np
import ml_dtypes

import concourse.bass as bass
import concourse.tile as tile
from concourse import bacc, mybir
from concourse.bass_utils import run_bass_kernel_spmd
from concourse.masks import make_identity

BF16 = ml_dtypes.bfloat16
DT = mybir.dt.bfloat16
F32 = mybir.dt.float32

T, DIM, HEADS, KVH, HD = 4096, 2048, 16, 4, 128
NCORES = 8
QH = HEADS // NCORES            # q heads per core = 2
WCOLS = QH * HD + 2 * HD        # wqkv cols per core = 512
TT = 512                        # query tile (psum bank free dim)
NTT = T // TT                   # 8
NSB = T // 128                  # 32 key blocks
TG = 1024                       # phase-1 token group
NTG = T // TG                   # 4
DBLK = DIM // 128               # 16 contraction blocks


def _block_structure(seq_ids):
    """Per query-tile list of allowed 128-key blocks, with masks for the
    partially-allowed ones. Block orientation matches psum_S: [s, t]."""
    seg = np.asarray(seq_ids).astype(np.int64)
    idx = np.arange(T)
    allowed = (seg[:, None] == seg[None, :]) & (idx[:, None] <= idx[None, :])
    block_list, masks = [], []
    for tt in range(NTT):
        t0 = tt * TT
        lst = []
        for sb in range(NSB):
            s0 = sb * 128
            blk = allowed[s0:s0 + 128, t0:t0 + TT]
            if not blk.any():
                continue
            if blk.all():
                lst.append((sb, None))
            else:
                masks.append(blk)
                lst.append((sb, len(masks) - 1))
        block_list.append(lst)
    if masks:
        masks_arr = np.stack(masks).astype(BF16)
    else:
        masks_arr = np.zeros((1, 128, TT), BF16)
    return block_list, masks_arr


def _build_program(block_list, n_masks):
    nc = bacc.Bacc("TRN2", target_bir_lowering=False, debug=False,
                   num_devices=NCORES)
    xT_d = nc.dram_tensor("xT", [DIM, T], DT, kind="ExternalInput")
    wqkv_d = nc.dram_tensor("wqkv", [DIM, WCOLS], DT, kind="ExternalInput")
    wo_d = nc.dram_tensor("wo", [DIM, DIM], DT, kind="ExternalInput")
    cos2_d = nc.dram_tensor("cos2", [HD, T], DT, kind="ExternalInput")
    sin2_d = nc.dram_tensor("sin2", [HD, T], DT, kind="ExternalInput")
    p64_d = nc.dram_tensor("p64", [HD, HD], DT, kind="ExternalInput")
    masks_d = nc.dram_tensor("masks", [n_masks, 128, TT], DT,
                             kind="ExternalInput")
    out_d = nc.dram_tensor("out", [TT, DIM], F32, kind="ExternalOutput")

    EXP = mybir.ActivationFunctionType.Exp
    COPY = mybir.ActivationFunctionType.Copy

    with tile.TileContext(nc) as tc:
        with tc.tile_pool(name="persist", bufs=1) as persist, \
             tc.tile_pool(name="p3early", bufs=1) as p3e, \
             tc.tile_pool(name="dram", bufs=1, space="DRAM") as dram:
            QT0 = persist.tile([HD, T], DT, name="QT0")
            QT1 = persist.tile([HD, T], DT, name="QT1")
            QT = [QT0, QT1]
            KT = persist.tile([HD, T], DT, name="KT")
            Vn = persist.tile([HD, T], DT, name="Vn")
            ones_sb = persist.tile([128, 128], DT, name="ones_sb")
            nc.vector.memset(ones_sb[:], 1.0)
            ident = persist.tile([128, 128], DT, name="ident")
            make_identity(nc, ident[:])
            p64_sb = persist.tile([HD, HD], DT, name="p64_sb")
            nc.scalar.dma_start(out=p64_sb[:], in_=p64_d[:])
            # weights as lhsT tiles: w_sb[p, d, j] = wqkv[d*128+p, j]
            # (split per d-block so the first matmul starts early)
            w_sb = persist.tile([128, DBLK, WCOLS], DT, name="w_sb")
            for d in range(DBLK):
                nc.scalar.dma_start(
                    out=w_sb[:, d, :],
                    in_=wqkv_d[d * 128:(d + 1) * 128, :])

            # chunked all-to-all: one exchange per token group. A2A-g's
            # chunk c is this core's attention output for columns
            # [g*1024 + c*128, +128); dest c therefore receives
            # attTfull[:, g*1024 + c*128 : +128] and finally owns tokens
            # {g*1024 + c*128 + [0,128) : g in 0..3}.
            attT_perm = [dram.tile([NCORES, QH * HD, 128], DT,
                                   name=f"attT_perm{g}") for g in range(NTG)]
            a2a_out = [dram.tile([DIM, 128], DT, name=f"a2a_out{g}")
                       for g in range(NTG)]

            # ---------------- phase 1 + 2: projection, rope, attention ----
            with tc.tile_pool(name="p1sbuf", bufs=1) as p1s, \
                 tc.tile_pool(name="xpool", bufs=18) as xpool, \
                 tc.tile_pool(name="p1tmp", bufs=3) as p1t, \
                 tc.tile_pool(name="p1psum", bufs=1, space="PSUM") as p1p, \
                 tc.tile_pool(name="atpsum", bufs=1, space="PSUM") as atp, \
                 tc.tile_pool(name="atsbuf", bufs=1) as ats:
                cos_sb = p1s.tile([HD, T], DT, name="cos_sb")
                nc.scalar.dma_start(out=cos_sb[:], in_=cos2_d[:])
                sin_sb = p1s.tile([HD, T], DT, name="sin_sb")
                nc.scalar.dma_start(out=sin_sb[:], in_=sin2_d[:])

                # wqkv column groups in processing order: k, v, q0, q1
                JSLICE = {"q0": 0, "q1": HD, "k": QH * HD, "v": QH * HD + HD}
                attS = [[] for _ in range(NTG)]
                for tg in range(NTG):
                    g0 = tg * TG
                    xt = []
                    for d in range(DBLK):
                        xtile = xpool.tile([128, TG], DT, name="xtile",
                                           bufs=18)
                        nc.sync.dma_start(
                            out=xtile[:],
                            in_=xT_d[d * 128:(d + 1) * 128, g0:g0 + TG])
                        xt.append(xtile)
                    for jname in ("k", "v", "q0", "q1"):
                        j0 = JSLICE[jname]
                        for th in range(TG // TT):
                            c0 = g0 + th * TT
                            pp = p1p.tile([128, TT], F32, name="pp", bufs=2)
                            for d in range(DBLK):
                                nc.tensor.matmul(
                                    pp[:],
                                    lhsT=w_sb[:, d, j0:j0 + HD],
                                    rhs=xt[d][:, th * TT:(th + 1) * TT],
                                    start=(d == 0), stop=(d == DBLK - 1))
                            if jname == "v":
                                vt_tmp = p1t.tile([128, TT], DT,
                                                  name="vt_tmp")
                                nc.vector.tensor_copy(vt_tmp[:], pp[:])
                                for i in range(TT // 128):
                                    ptr = p1p.tile([128, 128], DT,
                                                   name="ptmp", tag="ptmp",
                                                   bufs=2,
                                                   padded_shape=[128, 512])
                                    nc.tensor.transpose(
                                        ptr[:],
                                        vt_tmp[:, i * 128:(i + 1) * 128],
                                        ident[:])
                                    s0 = c0 + i * 128
                                    nc.vector.tensor_copy(
                                        Vn[:, s0:s0 + 128], ptr[:])
                            else:
                                dst = {"k": KT, "q0": QT0, "q1": QT1}[jname]
                                raw = p1t.tile([128, TT], DT, name="raw")
                                nc.vector.tensor_copy(raw[:], pp[:])
                                psw = p1p.tile([128, TT], F32, name="psw",
                                               tag="ptmp", bufs=2)
                                nc.tensor.matmul(psw[:], lhsT=p64_sb[:],
                                                 rhs=raw[:],
                                                 start=True, stop=True)
                                t1 = p1t.tile([128, TT], DT, name="t1")
                                nc.vector.tensor_mul(
                                    t1[:], raw[:], cos_sb[:, c0:c0 + TT])
                                t2 = p1t.tile([128, TT], DT, name="t2")
                                nc.vector.tensor_mul(
                                    t2[:], psw[:], sin_sb[:, c0:c0 + TT])
                                nc.vector.tensor_add(
                                    dst[:, c0:c0 + TT], t1[:], t2[:])

                    # -------- attention for this token group's query tiles
                    for tt in (2 * tg, 2 * tg + 1):
                        for h in range(QH):
                            t0 = tt * TT
                            blocks = block_list[tt]
                            nb = len(blocks)
                            pOT = atp.tile([128, TT], F32, name="pOT",
                                           bufs=1)
                            pSUM = atp.tile([128, TT], F32, name="pSUM",
                                            bufs=1)
                            for bi, (sb, mi) in enumerate(blocks):
                                s0 = sb * 128
                                pS = atp.tile([128, TT], F32, name="pS",
                                              bufs=2)
                                nc.tensor.matmul(
                                    pS[:], lhsT=KT[:, s0:s0 + 128],
                                    rhs=QT[h][:, t0:t0 + TT],
                                    start=True, stop=True)
                                expS = ats.tile([128, TT], DT, name="expS",
                                                bufs=4)
                                nc.scalar.activation(expS[:], pS[:], EXP)
                                if mi is not None:
                                    mt = ats.tile([128, TT], DT, name="mt",
                                                  bufs=3)
                                    nc.sync.dma_start(out=mt[:],
                                                      in_=masks_d[mi])
                                    expM = ats.tile([128, TT], DT,
                                                    name="expM", bufs=3)
                                    nc.vector.tensor_mul(expM[:], expS[:],
                                                         mt[:])
                                    expS = expM
                                nc.tensor.matmul(
                                    pSUM[:], lhsT=ones_sb[:], rhs=expS[:],
                                    start=(bi == 0), stop=(bi == nb - 1))
                                nc.tensor.matmul(
                                    pOT[:], lhsT=Vn[:, s0:s0 + 128],
                                    rhs=expS[:],
                                    start=(bi == 0), stop=(bi == nb - 1))
                            recip = ats.tile([128, TT], F32, name="recip",
                                             bufs=2)
                            nc.vector.reciprocal_approx_fast(
                                out=recip[:], in_=pSUM[:])
                            tmpn = ats.tile([128, TT], DT, name="tmpn",
                                            bufs=3)
                            nc.vector.tensor_mul(tmpn[:], pOT[:], recip[:])
                            c0 = (tt % 2) * 4
                            nc.sync.dma_start(
                                out=attT_perm[tg][c0:c0 + 4,
                                                  h * HD:(h + 1) * HD, :]
                                .rearrange("c p w -> p c w"),
                                in_=tmpn[:].rearrange("p (c w) -> p c w",
                                                      c=4))

                    # fire this token group's all-to-all under the next
                    # group's compute, and fetch its slices back
                    nc.gpsimd.collective_compute(
                        "AllToAll", mybir.AluOpType.bypass,
                        replica_groups=[list(range(NCORES))],
                        ins=[attT_perm[tg][:].opt()],
                        outs=[a2a_out[tg][:].opt()])
                    for jb in range(DBLK):
                        a_t = p3e.tile([128, 128], DT, name="attS",
                                       bufs=4 * DBLK)
                        nc.gpsimd.dma_start(
                            out=a_t[:],
                            in_=a2a_out[tg][jb * 128:(jb + 1) * 128, :])
                        attS[tg].append(a_t)

            # ---------------- output projection --------------------------
            # og-half outer with wo rows resident, g inner: each token
            # block's accumulation finishes on its own jb sweep so the
            # psum drain overlaps the next block's matmuls.
            with tc.tile_pool(name="p3psum", bufs=1, space="PSUM") as p3p:
                OG = 1024
                for og in range(DIM // OG):
                    wo_res = []
                    for jb in range(DBLK):
                        wo_t = p3e.tile([128, OG], DT, name="wo_t",
                                        bufs=DBLK)
                        nc.sync.dma_start(
                            out=wo_t[:],
                            in_=wo_d[jb * 128:(jb + 1) * 128,
                                     og * OG:(og + 1) * OG])
                        wo_res.append(wo_t)
                    for g in range(NTG):
                        po = p3p.tile([128, OG], F32, name="po", bufs=3)
                        for jb in range(DBLK):
                            for ods in range(OG // TT):
                                nc.tensor.matmul(
                                    po[:, ods * TT:(ods + 1) * TT],
                                    lhsT=attS[g][jb][:],
                                    rhs=wo_res[jb][:, ods * TT:(ods + 1) * TT],
                                    start=(jb == 0), stop=(jb == DBLK - 1))
                        ot = p3e.tile([128, OG], F32, name="ot", bufs=4)
                        if g % 2 == 0:
                            nc.vector.tensor_copy(ot[:], po[:])
                        else:
                            nc.scalar.activation(ot[:], po[:], COPY)
                        nc.sync.dma_start(
                            out=out_d[g * 128:(g + 1) * 128,
                                      og * OG:(og + 1) * OG],
                            in_=ot[:])

    nc.compile()
    return nc


def _prep_inputs(x, wq, wk, wv, wo, freqs_cos, freqs_sin):
    """Host-side transforms; returns the per-core in_maps."""
    perm = np.concatenate([np.arange(0, HD, 2), np.arange(1, HD, 2)])
    scale = 1.0 / np.sqrt(HD)
    # per-head de-interleave permutation of wq / wk columns
    wq_p = wq.reshape(DIM, HEADS, HD)[:, :, perm] * scale   # [DIM, 16, 128]
    wk_p = wk.reshape(DIM, KVH, HD)[:, :, perm]             # [DIM, 4, 128]
    wv_r = wv.reshape(DIM, KVH, HD)                         # [DIM, 4, 128]

    xT = np.ascontiguousarray(x.T).astype(BF16)
    wo_b = np.ascontiguousarray(wo).astype(BF16)

    cosT = np.ascontiguousarray(freqs_cos.T)                # [64, T]
    sinT = np.ascontiguousarray(freqs_sin.T)
    cos2 = np.concatenate([cosT, cosT], axis=0).astype(BF16)   # [128, T]
    sin2 = np.concatenate([-sinT, sinT], axis=0).astype(BF16)
    p64 = np.zeros((HD, HD), np.float32)
    p64[(np.arange(HD) + 64) % HD, np.arange(HD)] = 1.0
    p64 = p64.astype(BF16)

    in_maps = []
    for c in range(NCORES):
        g = c // 2
        wqkv = np.concatenate(
            [wq_p[:, 2 * c], wq_p[:, 2 * c + 1], wk_p[:, g], wv_r[:, g]],
            axis=1).astype(BF16)                             # [DIM, 512]
        in_maps.append({
            "xT": xT, "wqkv": np.ascontiguousarray(wqkv), "wo": wo_b,
            "cos2": cos2, "sin2": sin2, "p64": p64,
        })
    return in_maps


def kernel(x, wq, wk, wv, wo, freqs_cos, freqs_sin, seq_ids):
    x = np.asarray(x, np.float32)
    wq = np.asarray(wq, np.float32)
    wk = np.asarray(wk, np.float32)
    wv = np.asarray(wv, np.float32)
    wo = np.asarray(wo, np.float32)
    freqs_cos = np.asarray(freqs_cos, np.float32)
    freqs_sin = np.asarray(freqs_sin, np.float32)
    seq_ids = np.asarray(seq_ids)

    block_list, masks_arr = _block_structure(seq_ids)
    nc = _build_program(block_list, masks_arr.shape[0])
    in_maps = _prep_inputs(x, wq, wk, wv, wo, freqs_cos, freqs_sin)
    for m in in_maps:
        m["masks"] = masks_arr

    trace = bool(os.environ.get("BASS_KERNEL_TRACE"))
    if trace:
        sys.path.insert(0, "/root/problem")
        import axon_shim
        axon_shim.install()
    res = None
    for attempt in range(3):
        try:
            res = run_bass_kernel_spmd(
                nc, in_maps, core_ids=list(range(NCORES)), trace=trace)
            break
        except Exception:
            if attempt == 2:
                raise
            import time as _time
            import jax as _jax
            _jax.clear_caches()
            _time.sleep(5)
    if trace:
        print(f"HW exec time: {res.exec_time_ns} ns")
        kernel.last_exec_time_ns = res.exec_time_ns
        kernel.last_results = res
    out = np.empty((T, DIM), np.float32)
    for c in range(NCORES):
        oc = res.results[c]["out"]
        for g in range(NTG):
            out[g * TG + c * 128:g * TG + (c + 1) * 128] = \
                oc[g * 128:(g + 1) * 128]
    return out


# revision 18
# speedup vs baseline: 1.0970x; 1.0970x over previous
"""GQA attention (16 Q heads / 4 KV heads, head_dim 128, RoPE, varlen causal)
on 8 Trainium2 NeuronCores, tensor-parallel over heads.

Per core c: Q heads {2c, 2c+1}, KV head c//2.
Pipeline: QKV projection (bf16 matmul, f32 PSUM) -> RoPE (permutation-matmul
swap + DVE combine) -> block-sparse S^T-layout attention (exp on ScalarE,
softmax denominators via ones-matmul, PV accumulated directly in O^T layout)
-> normalize (fast DVE reciprocal) -> AllToAll (each core receives the full
attention output for its 512-token slice) -> output projection on that slice.

Host-side prep: x shipped pre-transposed; 1/sqrt(HD) folded into wq; wq/wk
columns permuted per head so RoPE's interleaved pairs become [evens | odds]
(dot products are permutation-invariant); varlen-causal mask block structure
computed from seq_ids and baked into the (shared, SPMD) program, with
multiplicative {0,1} masks shipped only for partially-masked blocks.
"""
import os
import sys

for _p in ("/opt/trn_rl_repo",):
    if _p not in sys.path:
        sys.path.insert(0, _p)

import numpy as np
import ml_dtypes

import concourse.bass as bass
import concourse.tile as tile
from concourse import bacc, mybir
from concourse.bass_utils import run_bass_kernel_spmd
from concourse.masks import make_identity

BF16 = ml_dtypes.bfloat16
DT = mybir.dt.bfloat16
F32 = mybir.dt.float32

T, DIM, HEADS, KVH, HD = 4096, 2048, 16, 4, 128
NCORES = 8
QH = HEADS // NCORES            # q heads per core = 2
WCOLS = QH * HD + 2 * HD        # wqkv cols per core = 512
TT = 512                        # query tile (psum bank free dim)
NTT = T // TT                   # 8
NSB = T // 128                  # 32 key blocks
TG = 1024                       # phase-1 token group
NTG = T // TG                   # 4
DBLK = DIM // 128               # 16 contraction blocks


def _block_structure(seq_ids):
    """Per query-tile list of allowed 128-key blocks, with masks for the
    partially-allowed ones. Block orientation matches psum_S: [s, t]."""
    seg = np.asarray(seq_ids).astype(np.int64)
    idx = np.arange(T)
    allowed = (seg[:, None] == seg[None, :]) & (idx[:, None] <= idx[None, :])
    block_list, masks = [], []
    for tt in range(NTT):
        t0 = tt * TT
        lst = []
        for sb in range(NSB):
            s0 = sb * 128
            blk = allowed[s0:s0 + 128, t0:t0 + TT]
            if not blk.any():
                continue
            if blk.all():
                lst.append((sb, None))
            else:
                masks.append(blk)
                lst.append((sb, len(masks) - 1))
        block_list.append(lst)
    if masks:
        masks_arr = np.stack(masks).astype(BF16)
    else:
        masks_arr = np.zeros((1, 128, TT), BF16)
    return block_list, masks_arr


def _build_program(block_list, n_masks):
    nc = bacc.Bacc("TRN2", target_bir_lowering=False, debug=False,
                   num_devices=NCORES)
    xT_d = nc.dram_tensor("xT", [DIM, T], DT, kind="ExternalInput")
    wqkv_d = nc.dram_tensor("wqkv", [DIM, WCOLS], DT, kind="ExternalInput")
    wo_d = nc.dram_tensor("wo", [DIM, DIM], DT, kind="ExternalInput")
    cos2_d = nc.dram_tensor("cos2", [HD, T], DT, kind="ExternalInput")
    sin2_d = nc.dram_tensor("sin2", [HD, T], DT, kind="ExternalInput")
    p64_d = nc.dram_tensor("p64", [HD, HD], DT, kind="ExternalInput")
    masks_d = nc.dram_tensor("masks", [n_masks, 128, TT], DT,
                             kind="ExternalInput")
    out_d = nc.dram_tensor("out", [TT, DIM], F32, kind="ExternalOutput")

    EXP = mybir.ActivationFunctionType.Exp
    COPY = mybir.ActivationFunctionType.Copy

    with tile.TileContext(nc) as tc:
        with tc.tile_pool(name="persist", bufs=1) as persist, \
             tc.tile_pool(name="p3early", bufs=1) as p3e, \
             tc.tile_pool(name="dram", bufs=1, space="DRAM") as dram:
            QT0 = persist.tile([HD, T], DT, name="QT0")
            QT1 = persist.tile([HD, T], DT, name="QT1")
            QT = [QT0, QT1]
            KT = persist.tile([HD, T], DT, name="KT")
            Vn = persist.tile([HD, T], DT, name="Vn")
            ones_sb = persist.tile([128, 128], DT, name="ones_sb")
            nc.vector.memset(ones_sb[:], 1.0)
            ident = persist.tile([128, 128], DT, name="ident")
            make_identity(nc, ident[:])
            p64_sb = persist.tile([HD, HD], DT, name="p64_sb")
            nc.scalar.dma_start(out=p64_sb[:], in_=p64_d[:])
            # weights as lhsT tiles: w_sb[p, d, j] = wqkv[d*128+p, j]
            # (split per d-block so the first matmul starts early)
            w_sb = persist.tile([128, DBLK, WCOLS], DT, name="w_sb")
            for d in range(DBLK):
                nc.scalar.dma_start(
                    out=w_sb[:, d, :],
                    in_=wqkv_d[d * 128:(d + 1) * 128, :])

            # chunked all-to-all: one exchange per token group. A2A-g's
            # chunk c is this core's attention output for columns
            # [g*1024 + c*128, +128); dest c therefore receives
            # attTfull[:, g*1024 + c*128 : +128] and finally owns tokens
            # {g*1024 + c*128 + [0,128) : g in 0..3}.
            attT_perm = [dram.tile([NCORES, QH * HD, 128], DT,
                                   name=f"attT_perm{g}") for g in range(NTG)]
            a2a_out = [dram.tile([DIM, 128], DT, name=f"a2a_out{g}")
                       for g in range(NTG)]

            # ---------------- phase 1 + 2: projection, rope, attention ----
            with tc.tile_pool(name="p1sbuf", bufs=1) as p1s, \
                 tc.tile_pool(name="xpool", bufs=22) as xpool, \
                 tc.tile_pool(name="p1tmp", bufs=3) as p1t, \
                 tc.tile_pool(name="p1psum", bufs=1, space="PSUM") as p1p, \
                 tc.tile_pool(name="atpsum", bufs=1, space="PSUM") as atp, \
                 tc.tile_pool(name="atsbuf", bufs=1) as ats:
                cos_sb = p1s.tile([HD, T], DT, name="cos_sb")
                nc.scalar.dma_start(out=cos_sb[:], in_=cos2_d[:])
                sin_sb = p1s.tile([HD, T], DT, name="sin_sb")
                nc.scalar.dma_start(out=sin_sb[:], in_=sin2_d[:])

                # wqkv column groups in processing order: k, v, q0, q1
                JSLICE = {"q0": 0, "q1": HD, "k": QH * HD, "v": QH * HD + HD}
                attS = [[] for _ in range(NTG)]
                for tg in range(NTG):
                    g0 = tg * TG
                    xt = []
                    for d in range(DBLK):
                        xtile = xpool.tile([128, TG], DT, name="xtile",
                                           bufs=22)
                        eng = nc.sync if d % 2 == 0 else nc.scalar
                        eng.dma_start(
                            out=xtile[:],
                            in_=xT_d[d * 128:(d + 1) * 128, g0:g0 + TG])
                        xt.append(xtile)
                    for jname in ("k", "v", "q0", "q1"):
                        j0 = JSLICE[jname]
                        for th in range(TG // TT):
                            c0 = g0 + th * TT
                            pp = p1p.tile([128, TT], F32, name="pp", bufs=2)
                            for d in range(DBLK):
                                nc.tensor.matmul(
                                    pp[:],
                                    lhsT=w_sb[:, d, j0:j0 + HD],
                                    rhs=xt[d][:, th * TT:(th + 1) * TT],
                                    start=(d == 0), stop=(d == DBLK - 1))
                            if jname == "v":
                                vt_tmp = p1t.tile([128, TT], DT,
                                                  name="vt_tmp")
                                nc.vector.tensor_copy(vt_tmp[:], pp[:])
                                for i in range(TT // 128):
                                    ptr = p1p.tile([128, 128], DT,
                                                   name="ptmp", tag="ptmp",
                                                   bufs=2,
                                                   padded_shape=[128, 512])
                                    nc.tensor.transpose(
                                        ptr[:],
                                        vt_tmp[:, i * 128:(i + 1) * 128],
                                        ident[:])
                                    s0 = c0 + i * 128
                                    nc.vector.tensor_copy(
                                        Vn[:, s0:s0 + 128], ptr[:])
                            else:
                                dst = {"k": KT, "q0": QT0, "q1": QT1}[jname]
                                raw = p1t.tile([128, TT], DT, name="raw")
                                nc.vector.tensor_copy(raw[:], pp[:])
                                psw = p1p.tile([128, TT], F32, name="psw",
                                               tag="ptmp", bufs=2)
                                nc.tensor.matmul(psw[:], lhsT=p64_sb[:],
                                                 rhs=raw[:],
                                                 start=True, stop=True)
                                t1 = p1t.tile([128, TT], DT, name="t1")
                                nc.vector.tensor_mul(
                                    t1[:], raw[:], cos_sb[:, c0:c0 + TT])
                                t2 = p1t.tile([128, TT], DT, name="t2")
                                nc.vector.tensor_mul(
                                    t2[:], psw[:], sin_sb[:, c0:c0 + TT])
                                nc.vector.tensor_add(
                                    dst[:, c0:c0 + TT], t1[:], t2[:])

                    # -------- attention for this token group's query tiles
                    for tt in (2 * tg, 2 * tg + 1):
                        for h in range(QH):
                            t0 = tt * TT
                            blocks = block_list[tt]
                            nb = len(blocks)
                            pOT = atp.tile([128, TT], F32, name="pOT",
                                           bufs=1)
                            pSUM = atp.tile([128, TT], F32, name="pSUM",
                                            bufs=1)
                            for bi, (sb, mi) in enumerate(blocks):
                                s0 = sb * 128
                                pS = atp.tile([128, TT], F32, name="pS",
                                              bufs=2)
                                nc.tensor.matmul(
                                    pS[:], lhsT=KT[:, s0:s0 + 128],
                                    rhs=QT[h][:, t0:t0 + TT],
                                    start=True, stop=True)
                                expS = ats.tile([128, TT], DT, name="expS",
                                                bufs=4)
                                nc.scalar.activation(expS[:], pS[:], EXP)
                                if mi is not None:
                                    mt = ats.tile([128, TT], DT, name="mt",
                                                  bufs=3)
                                    nc.sync.dma_start(out=mt[:],
                                                      in_=masks_d[mi])
                                    expM = ats.tile([128, TT], DT,
                                                    name="expM", bufs=3)
                                    nc.vector.tensor_mul(expM[:], expS[:],
                                                         mt[:])
                                    expS = expM
                                nc.tensor.matmul(
                                    pSUM[:], lhsT=ones_sb[:], rhs=expS[:],
                                    start=(bi == 0), stop=(bi == nb - 1))
                                nc.tensor.matmul(
                                    pOT[:], lhsT=Vn[:, s0:s0 + 128],
                                    rhs=expS[:],
                                    start=(bi == 0), stop=(bi == nb - 1))
                            recip = ats.tile([128, TT], F32, name="recip",
                                             bufs=2)
                            nc.vector.reciprocal_approx_fast(
                                out=recip[:], in_=pSUM[:])
                            tmpn = ats.tile([128, TT], DT, name="tmpn",
                                            bufs=3)
                            nc.vector.tensor_mul(tmpn[:], pOT[:], recip[:])
                            c0 = (tt % 2) * 4
                            nc.sync.dma_start(
                                out=attT_perm[tg][c0:c0 + 4,
                                                  h * HD:(h + 1) * HD, :]
                                .rearrange("c p w -> p c w"),
                                in_=tmpn[:].rearrange("p (c w) -> p c w",
                                                      c=4))

                    # fire this token group's all-to-all under the next
                    # group's compute, and fetch its slices back
                    nc.gpsimd.collective_compute(
                        "AllToAll", mybir.AluOpType.bypass,
                        replica_groups=[list(range(NCORES))],
                        ins=[attT_perm[tg][:].opt()],
                        outs=[a2a_out[tg][:].opt()])
                    a_g = p3e.tile([128, DBLK, 128], DT, name="attS",
                                   bufs=NTG)
                    nc.gpsimd.dma_start(
                        out=a_g[:],
                        in_=a2a_out[tg][:].rearrange("(jb p) w -> p jb w",
                                                     p=128))
                    attS[tg] = a_g

            # ---------------- output projection --------------------------
            # og-half outer with wo rows resident, g inner: each token
            # block's accumulation finishes on its own jb sweep so the
            # psum drain overlaps the next block's matmuls.
            with tc.tile_pool(name="p3psum", bufs=1, space="PSUM") as p3p:
                OG = 1024
                for og in range(DIM // OG):
                    wo_res = []
                    for jb in range(DBLK):
                        wo_t = p3e.tile([128, OG], DT, name="wo_t",
                                        bufs=DBLK)
                        nc.sync.dma_start(
                            out=wo_t[:],
                            in_=wo_d[jb * 128:(jb + 1) * 128,
                                     og * OG:(og + 1) * OG])
                        wo_res.append(wo_t)
                    for g in range(NTG):
                        po = p3p.tile([128, OG], F32, name="po", bufs=3)
                        for jb in range(DBLK):
                            for ods in range(OG // TT):
                                nc.tensor.matmul(
                                    po[:, ods * TT:(ods + 1) * TT],
                                    lhsT=attS[g][:, jb, :],
                                    rhs=wo_res[jb][:, ods * TT:(ods + 1) * TT],
                                    start=(jb == 0), stop=(jb == DBLK - 1))
                        ot = p3e.tile([128, OG], F32, name="ot", bufs=4)
                        if g % 2 == 0:
                            nc.vector.tensor_copy(ot[:], po[:])
                        else:
                            nc.scalar.activation(ot[:], po[:], COPY)
                        nc.sync.dma_start(
                            out=out_d[g * 128:(g + 1) * 128,
                                      og * OG:(og + 1) * OG],
                            in_=ot[:])

    nc.compile()
    return nc


def _prep_inputs(x, wq, wk, wv, wo, freqs_cos, freqs_sin):
    """Host-side transforms; returns the per-core in_maps."""
    perm = np.concatenate([np.arange(0, HD, 2), np.arange(1, HD, 2)])
    scale = 1.0 / np.sqrt(HD)
    # per-head de-interleave permutation of wq / wk columns
    wq_p = wq.reshape(DIM, HEADS, HD)[:, :, perm] * scale   # [DIM, 16, 128]
    wk_p = wk.reshape(DIM, KVH, HD)[:, :, perm]             # [DIM, 4, 128]
    wv_r = wv.reshape(DIM, KVH, HD)                         # [DIM, 4, 128]

    xT = np.ascontiguousarray(x.T).astype(BF16)
    wo_b = np.ascontiguousarray(wo).astype(BF16)

    cosT = np.ascontiguousarray(freqs_cos.T)                # [64, T]
    sinT = np.ascontiguousarray(freqs_sin.T)
    cos2 = np.concatenate([cosT, cosT], axis=0).astype(BF16)   # [128, T]
    sin2 = np.concatenate([-sinT, sinT], axis=0).astype(BF16)
    p64 = np.zeros((HD, HD), np.float32)
    p64[(np.arange(HD) + 64) % HD, np.arange(HD)] = 1.0
    p64 = p64.astype(BF16)

    in_maps = []
    for c in range(NCORES):
        g = c // 2
        wqkv = np.concatenate(
            [wq_p[:, 2 * c], wq_p[:, 2 * c + 1], wk_p[:, g], wv_r[:, g]],
            axis=1).astype(BF16)                             # [DIM, 512]
        in_maps.append({
            "xT": xT, "wqkv": np.ascontiguousarray(wqkv), "wo": wo_b,
            "cos2": cos2, "sin2": sin2, "p64": p64,
        })
    return in_maps


def kernel(x, wq, wk, wv, wo, freqs_cos, freqs_sin, seq_ids):
    x = np.asarray(x, np.float32)
    wq = np.asarray(wq, np.float32)
    wk = np.asarray(wk, np.float32)
    wv = np.asarray(wv, np.float32)
    wo = np.asarray(wo, np.float32)
    freqs_cos = np.asarray(freqs_cos, np.float32)
    freqs_sin = np.asarray(freqs_sin, np.float32)
    seq_ids = np.asarray(seq_ids)

    block_list, masks_arr = _block_structure(seq_ids)
    nc = _build_program(block_list, masks_arr.shape[0])
    in_maps = _prep_inputs(x, wq, wk, wv, wo, freqs_cos, freqs_sin)
    for m in in_maps:
        m["masks"] = masks_arr

    trace = bool(os.environ.get("BASS_KERNEL_TRACE"))
    if trace:
        sys.path.insert(0, "/root/problem")
        import axon_shim
        axon_shim.install()
    res = None
    for attempt in range(3):
        try:
            res = run_bass_kernel_spmd(
                nc, in_maps, core_ids=list(range(NCORES)), trace=trace)
            break
        except Exception:
            if attempt == 2:
                raise
            import time as _time
            import jax as _jax
            _jax.clear_caches()
            _time.sleep(5)
    if trace:
        print(f"HW exec time: {res.exec_time_ns} ns")
        kernel.last_exec_time_ns = res.exec_time_ns
        kernel.last_results = res
    out = np.empty((T, DIM), np.float32)
    for c in range(NCORES):
        oc = res.results[c]["out"]
        for g in range(NTG):
            out[g * TG + c * 128:g * TG + (c + 1) * 128] = \
                oc[g * 128:(g + 1) * 128]
    return out


# revision 19
# speedup vs baseline: 1.1975x; 1.0917x over previous
"""GQA attention (16 Q heads / 4 KV heads, head_dim 128, RoPE, varlen causal)
on 8 Trainium2 NeuronCores, tensor-parallel over heads.

Per core c: Q heads {2c, 2c+1}, KV head c//2.
Pipeline: QKV projection (bf16 matmul, f32 PSUM) -> RoPE (permutation-matmul
swap + DVE combine) -> block-sparse S^T-layout attention (exp on ScalarE,
softmax denominators via ones-matmul, PV accumulated directly in O^T layout)
-> normalize (fast DVE reciprocal) -> AllToAll (each core receives the full
attention output for its 512-token slice) -> output projection on that slice.

Host-side prep: x shipped pre-transposed; 1/sqrt(HD) folded into wq; wq/wk
columns permuted per head so RoPE's interleaved pairs become [evens | odds]
(dot products are permutation-invariant); varlen-causal mask block structure
computed from seq_ids and baked into the (shared, SPMD) program, with
multiplicative {0,1} masks shipped only for partially-masked blocks.
"""
import os
import sys

for _p in ("/opt/trn_rl_repo",):
    if _p not in sys.path:
        sys.path.insert(0, _p)

import numpy as np
import ml_dtypes

import concourse.bass as bass
import concourse.tile as tile
from concourse import bacc, mybir
from concourse.bass_utils import run_bass_kernel_spmd
from concourse.masks import make_identity

BF16 = ml_dtypes.bfloat16
DT = mybir.dt.bfloat16
F32 = mybir.dt.float32

T, DIM, HEADS, KVH, HD = 4096, 2048, 16, 4, 128
NCORES = 8
QH = HEADS // NCORES            # q heads per core = 2
WCOLS = QH * HD + 2 * HD        # wqkv cols per core = 512
TT = 512                        # query tile (psum bank free dim)
NTT = T // TT                   # 8
NSB = T // 128                  # 32 key blocks
TG = 1024                       # phase-1 token group
NTG = T // TG                   # 4
DBLK = DIM // 128               # 16 contraction blocks


def _block_structure(seq_ids):
    """Per query-tile list of allowed 128-key blocks, with masks for the
    partially-allowed ones. Block orientation matches psum_S: [s, t]."""
    seg = np.asarray(seq_ids).astype(np.int64)
    idx = np.arange(T)
    allowed = (seg[:, None] == seg[None, :]) & (idx[:, None] <= idx[None, :])
    block_list, masks = [], []
    for tt in range(NTT):
        t0 = tt * TT
        lst = []
        for sb in range(NSB):
            s0 = sb * 128
            blk = allowed[s0:s0 + 128, t0:t0 + TT]
            if not blk.any():
                continue
            if blk.all():
                lst.append((sb, None))
            else:
                masks.append(blk)
                lst.append((sb, len(masks) - 1))
        block_list.append(lst)
    if masks:
        masks_arr = np.stack(masks).astype(BF16)
    else:
        masks_arr = np.zeros((1, 128, TT), BF16)
    return block_list, masks_arr


def _build_program(block_list, n_masks):
    nc = bacc.Bacc("TRN2", target_bir_lowering=False, debug=False,
                   num_devices=NCORES)
    xT_d = nc.dram_tensor("xT", [DIM, T], DT, kind="ExternalInput")
    wqkv_d = nc.dram_tensor("wqkv", [DIM, WCOLS], DT, kind="ExternalInput")
    wo_d = nc.dram_tensor("wo", [DIM, DIM], DT, kind="ExternalInput")
    cos2_d = nc.dram_tensor("cos2", [HD, T], DT, kind="ExternalInput")
    sin2_d = nc.dram_tensor("sin2", [HD, T], DT, kind="ExternalInput")
    p64_d = nc.dram_tensor("p64", [HD, HD], DT, kind="ExternalInput")
    masks_d = nc.dram_tensor("masks", [n_masks, 128, TT], DT,
                             kind="ExternalInput")
    out_d = nc.dram_tensor("out", [TT, DIM], F32, kind="ExternalOutput")

    EXP = mybir.ActivationFunctionType.Exp
    COPY = mybir.ActivationFunctionType.Copy

    with tile.TileContext(nc) as tc:
        with tc.tile_pool(name="persist", bufs=1) as persist, \
             tc.tile_pool(name="p3early", bufs=1) as p3e, \
             tc.tile_pool(name="dram", bufs=1, space="DRAM") as dram:
            QT0 = persist.tile([HD, T], DT, name="QT0")
            QT1 = persist.tile([HD, T], DT, name="QT1")
            QT = [QT0, QT1]
            KT = persist.tile([HD, T], DT, name="KT")
            Vn = persist.tile([HD, T], DT, name="Vn")
            ones_sb = persist.tile([128, 128], DT, name="ones_sb")
            nc.vector.memset(ones_sb[:], 1.0)
            ident = persist.tile([128, 128], DT, name="ident")
            make_identity(nc, ident[:])
            p64_sb = persist.tile([HD, HD], DT, name="p64_sb")
            nc.scalar.dma_start(out=p64_sb[:], in_=p64_d[:])
            # weights as lhsT tiles: w_sb[p, d, j] = wqkv[d*128+p, j]
            # (split per d-block so the first matmul starts early)
            w_sb = persist.tile([128, DBLK, WCOLS], DT, name="w_sb")
            for d in range(DBLK):
                nc.scalar.dma_start(
                    out=w_sb[:, d, :],
                    in_=wqkv_d[d * 128:(d + 1) * 128, :])

            # chunked all-to-all: one exchange per token group. A2A-g's
            # chunk c is this core's attention output for columns
            # [g*1024 + c*128, +128); dest c therefore receives
            # attTfull[:, g*1024 + c*128 : +128] and finally owns tokens
            # {g*1024 + c*128 + [0,128) : g in 0..3}.
            attT_perm = [dram.tile([NCORES, QH * HD, 128], DT,
                                   name=f"attT_perm{g}") for g in range(NTG)]
            a2a_out = [dram.tile([DIM, 128], DT, name=f"a2a_out{g}")
                       for g in range(NTG)]

            # ---------------- phase 1 + 2: projection, rope, attention ----
            with tc.tile_pool(name="p1sbuf", bufs=1) as p1s, \
                 tc.tile_pool(name="xpool", bufs=22) as xpool, \
                 tc.tile_pool(name="p1tmp", bufs=3) as p1t, \
                 tc.tile_pool(name="p1psum", bufs=1, space="PSUM") as p1p, \
                 tc.tile_pool(name="atpsum", bufs=1, space="PSUM") as atp, \
                 tc.tile_pool(name="atsbuf", bufs=1) as ats:
                cos_sb = p1s.tile([HD, T], DT, name="cos_sb")
                nc.scalar.dma_start(out=cos_sb[:], in_=cos2_d[:])
                sin_sb = p1s.tile([HD, T], DT, name="sin_sb")
                nc.scalar.dma_start(out=sin_sb[:], in_=sin2_d[:])

                # wqkv column groups in processing order: k, v, q0, q1
                JSLICE = {"q0": 0, "q1": HD, "k": QH * HD, "v": QH * HD + HD}
                attS = [[] for _ in range(NTG)]
                for tg in range(NTG):
                    g0 = tg * TG
                    xt = []
                    for d in range(DBLK):
                        xtile = xpool.tile([128, TG], DT, name="xtile",
                                           bufs=22)
                        nc.sync.dma_start(
                            out=xtile[:],
                            in_=xT_d[d * 128:(d + 1) * 128, g0:g0 + TG])
                        xt.append(xtile)
                    for jname in ("k", "v", "q0", "q1"):
                        j0 = JSLICE[jname]
                        for th in range(TG // TT):
                            c0 = g0 + th * TT
                            pp = p1p.tile([128, TT], F32, name="pp", bufs=2)
                            for d in range(DBLK):
                                nc.tensor.matmul(
                                    pp[:],
                                    lhsT=w_sb[:, d, j0:j0 + HD],
                                    rhs=xt[d][:, th * TT:(th + 1) * TT],
                                    start=(d == 0), stop=(d == DBLK - 1))
                            if jname == "v":
                                vt_tmp = p1t.tile([128, TT], DT,
                                                  name="vt_tmp")
                                nc.vector.tensor_copy(vt_tmp[:], pp[:])
                                for i in range(TT // 128):
                                    ptr = p1p.tile([128, 128], DT,
                                                   name="ptmp", tag="ptmp",
                                                   bufs=2,
                                                   padded_shape=[128, 512])
                                    nc.tensor.transpose(
                                        ptr[:],
                                        vt_tmp[:, i * 128:(i + 1) * 128],
                                        ident[:])
                                    s0 = c0 + i * 128
                                    nc.vector.tensor_copy(
                                        Vn[:, s0:s0 + 128], ptr[:])
                            else:
                                dst = {"k": KT, "q0": QT0, "q1": QT1}[jname]
                                raw = p1t.tile([128, TT], DT, name="raw")
                                nc.vector.tensor_copy(raw[:], pp[:])
                                psw = p1p.tile([128, TT], F32, name="psw",
                                               tag="ptmp", bufs=2)
                                nc.tensor.matmul(psw[:], lhsT=p64_sb[:],
                                                 rhs=raw[:],
                                                 start=True, stop=True)
                                t1 = p1t.tile([128, TT], DT, name="t1")
                                nc.vector.tensor_mul(
                                    t1[:], raw[:], cos_sb[:, c0:c0 + TT])
                                t2 = p1t.tile([128, TT], DT, name="t2")
                                nc.vector.tensor_mul(
                                    t2[:], psw[:], sin_sb[:, c0:c0 + TT])
                                nc.vector.tensor_add(
                                    dst[:, c0:c0 + TT], t1[:], t2[:])

                    # -------- attention for this token group's query tiles
                    for tt in (2 * tg, 2 * tg + 1):
                        for h in range(QH):
                            t0 = tt * TT
                            blocks = block_list[tt]
                            nb = len(blocks)
                            pOT = atp.tile([128, TT], F32, name="pOT",
                                           bufs=1)
                            pSUM = atp.tile([128, TT], F32, name="pSUM",
                                            bufs=1)
                            for bi, (sb, mi) in enumerate(blocks):
                                s0 = sb * 128
                                pS = atp.tile([128, TT], F32, name="pS",
                                              bufs=2)
                                nc.tensor.matmul(
                                    pS[:], lhsT=KT[:, s0:s0 + 128],
                                    rhs=QT[h][:, t0:t0 + TT],
                                    start=True, stop=True)
                                expS = ats.tile([128, TT], DT, name="expS",
                                                bufs=4)
                                nc.scalar.activation(expS[:], pS[:], EXP)
                                if mi is not None:
                                    mt = ats.tile([128, TT], DT, name="mt",
                                                  bufs=3)
                                    nc.sync.dma_start(out=mt[:],
                                                      in_=masks_d[mi])
                                    expM = ats.tile([128, TT], DT,
                                                    name="expM", bufs=3)
                                    nc.vector.tensor_mul(expM[:], expS[:],
                                                         mt[:])
                                    expS = expM
                                nc.tensor.matmul(
                                    pSUM[:], lhsT=ones_sb[:], rhs=expS[:],
                                    start=(bi == 0), stop=(bi == nb - 1))
                                nc.tensor.matmul(
                                    pOT[:], lhsT=Vn[:, s0:s0 + 128],
                                    rhs=expS[:],
                                    start=(bi == 0), stop=(bi == nb - 1))
                            recip = ats.tile([128, TT], F32, name="recip",
                                             bufs=2)
                            nc.vector.reciprocal_approx_fast(
                                out=recip[:], in_=pSUM[:])
                            tmpn = ats.tile([128, TT], DT, name="tmpn",
                                            bufs=3)
                            nc.vector.tensor_mul(tmpn[:], pOT[:], recip[:])
                            c0 = (tt % 2) * 4
                            nc.sync.dma_start(
                                out=attT_perm[tg][c0:c0 + 4,
                                                  h * HD:(h + 1) * HD, :]
                                .rearrange("c p w -> p c w"),
                                in_=tmpn[:].rearrange("p (c w) -> p c w",
                                                      c=4))

                    # fire this token group's all-to-all under the next
                    # group's compute, and fetch its slices back
                    nc.gpsimd.collective_compute(
                        "AllToAll", mybir.AluOpType.bypass,
                        replica_groups=[list(range(NCORES))],
                        ins=[attT_perm[tg][:].opt()],
                        outs=[a2a_out[tg][:].opt()])
                    a_g = p3e.tile([128, DBLK, 128], DT, name="attS",
                                   bufs=NTG)
                    nc.gpsimd.dma_start(
                        out=a_g[:],
                        in_=a2a_out[tg][:].rearrange("(jb p) w -> p jb w",
                                                     p=128))
                    attS[tg] = a_g

            # ---------------- output projection --------------------------
            # og-half outer with wo rows resident, g inner: each token
            # block's accumulation finishes on its own jb sweep so the
            # psum drain overlaps the next block's matmuls.
            with tc.tile_pool(name="p3psum", bufs=1, space="PSUM") as p3p:
                OG = 1024
                for og in range(DIM // OG):
                    wo_res = []
                    for jb in range(DBLK):
                        wo_t = p3e.tile([128, OG], DT, name="wo_t",
                                        bufs=DBLK)
                        nc.sync.dma_start(
                            out=wo_t[:],
                            in_=wo_d[jb * 128:(jb + 1) * 128,
                                     og * OG:(og + 1) * OG])
                        wo_res.append(wo_t)
                    for g in range(NTG):
                        po = p3p.tile([128, OG], F32, name="po", bufs=3)
                        for jb in range(DBLK):
                            for ods in range(OG // TT):
                                nc.tensor.matmul(
                                    po[:, ods * TT:(ods + 1) * TT],
                                    lhsT=attS[g][:, jb, :],
                                    rhs=wo_res[jb][:, ods * TT:(ods + 1) * TT],
                                    start=(jb == 0), stop=(jb == DBLK - 1))
                        ot = p3e.tile([128, OG], F32, name="ot", bufs=4)
                        if g % 2 == 0:
                            nc.vector.tensor_copy(ot[:], po[:])
                        else:
                            nc.scalar.activation(ot[:], po[:], COPY)
                        nc.sync.dma_start(
                            out=out_d[g * 128:(g + 1) * 128,
                                      og * OG:(og + 1) * OG],
                            in_=ot[:])

    nc.compile()
    return nc


def _prep_inputs(x, wq, wk, wv, wo, freqs_cos, freqs_sin):
    """Host-side transforms; returns the per-core in_maps."""
    perm = np.concatenate([np.arange(0, HD, 2), np.arange(1, HD, 2)])
    scale = 1.0 / np.sqrt(HD)
    # per-head de-interleave permutation of wq / wk columns
    wq_p = wq.reshape(DIM, HEADS, HD)[:, :, perm] * scale   # [DIM, 16, 128]
    wk_p = wk.reshape(DIM, KVH, HD)[:, :, perm]             # [DIM, 4, 128]
    wv_r = wv.reshape(DIM, KVH, HD)                         # [DIM, 4, 128]

    xT = np.ascontiguousarray(x.T).astype(BF16)
    wo_b = np.ascontiguousarray(wo).astype(BF16)

    cosT = np.ascontiguousarray(freqs_cos.T)                # [64, T]
    sinT = np.ascontiguousarray(freqs_sin.T)
    cos2 = np.concatenate([cosT, cosT], axis=0).astype(BF16)   # [128, T]
    sin2 = np.concatenate([-sinT, sinT], axis=0).astype(BF16)
    p64 = np.zeros((HD, HD), np.float32)
    p64[(np.arange(HD) + 64) % HD, np.arange(HD)] = 1.0
    p64 = p64.astype(BF16)

    in_maps = []
    for c in range(NCORES):
        g = c // 2
        wqkv = np.concatenate(
            [wq_p[:, 2 * c], wq_p[:, 2 * c + 1], wk_p[:, g], wv_r[:, g]],
            axis=1).astype(BF16)                             # [DIM, 512]
        in_maps.append({
            "xT": xT, "wqkv": np.ascontiguousarray(wqkv), "wo": wo_b,
            "cos2": cos2, "sin2": sin2, "p64": p64,
        })
    return in_maps


def kernel(x, wq, wk, wv, wo, freqs_cos, freqs_sin, seq_ids):
    x = np.asarray(x, np.float32)
    wq = np.asarray(wq, np.float32)
    wk = np.asarray(wk, np.float32)
    wv = np.asarray(wv, np.float32)
    wo = np.asarray(wo, np.float32)
    freqs_cos = np.asarray(freqs_cos, np.float32)
    freqs_sin = np.asarray(freqs_sin, np.float32)
    seq_ids = np.asarray(seq_ids)

    block_list, masks_arr = _block_structure(seq_ids)
    nc = _build_program(block_list, masks_arr.shape[0])
    in_maps = _prep_inputs(x, wq, wk, wv, wo, freqs_cos, freqs_sin)
    for m in in_maps:
        m["masks"] = masks_arr

    trace = bool(os.environ.get("BASS_KERNEL_TRACE"))
    if trace:
        sys.path.insert(0, "/root/problem")
        import axon_shim
        axon_shim.install()
    res = None
    for attempt in range(3):
        try:
            res = run_bass_kernel_spmd(
                nc, in_maps, core_ids=list(range(NCORES)), trace=trace)
            break
        except Exception:
            if attempt == 2:
                raise
            import time as _time
            import jax as _jax
            _jax.clear_caches()
            _time.sleep(5)
    if trace:
        print(f"HW exec time: {res.exec_time_ns} ns")
        kernel.last_exec_time_ns = res.exec_time_ns
        kernel.last_results = res
    out = np.empty((T, DIM), np.float32)
    for c in range(NCORES):
        oc = res.results[c]["out"]
        for g in range(NTG):
            out[g * TG + c * 128:g * TG + (c + 1) * 128] = \
                oc[g * 128:(g + 1) * 128]
    return out
